# revision 50
# baseline (speedup 1.0000x reference)
"""Bahdanau attention kernel for Trainium2 (Bass/Tile), 8-core data-parallel.

Problem shapes: B=32, Tx=1024, enc_hid=dec_hid=attn=1024, fp32 in/out.

Math (per example b):
  dec_proj = W_dec @ dec_hidden[b]                 [attn]
  energy^T[a, t] = tanh(sum_e W_enc[a,e] enc[b,t,e] + dec_proj[a] + W_b[a])
  scores[t] = sum_a v[a] energy^T[a, t]
  alpha = softmax(mask(scores))
  context[e] = sum_t alpha[t] enc[b,t,e]

Sharding: batch B split 4 examples per core across 8 cores; weights replicated.

All matmul operands are bf16: on TRN2 silicon a 512-moving-row fp32r matmul
measures ~394ns while bf16 measures ~260ns steady-state, and bf16 halves HBM
traffic and SBUF footprint. PSUM accumulation stays fp32. Measured fp22
end-to-end rel err was 2e-4; bf16's 8-bit mantissa puts it at ~3e-3, inside
the 2e-2 gate.

Masking is folded into the scores as an additive -100 penalty before a
single bf16 exp: masked lanes give exp(s-100) < 1e-40 which flushes to 0.0
in bf16 -- exactly the reference's masked_fill(-1e9) softmax behavior.

The context reduction runs on the VECTOR engine against the already-resident
encT tiles: alpha (bf16, pre-normalized) is partition-broadcast to [128, Tx]
and tensor_tensor_reduce contracts over t per 128-wide e-chunk. This deletes
the whole natural-layout enc DMA stream (8MB/core) and 64 context matmuls,
and leaves every SBUF tile resident for the kernel's lifetime (no slot
gating -> mid-kernel DMAs can never head-of-line block a compute queue).

Layouts per core (host-side preprocessing in kernel()):
  encT  [4, E, Tx]  bf16  enc transposed  -> e on partitions (energy rhs)
  w_encT [E, A], w_decT [D, A] bf16       transposed nn.Linear weights
  dec_hT [D, 4], v_col [A, 1] bf16, wb8 [128, 8] f32, mb [4, Tx] f32
"""

from contextlib import ExitStack

import numpy as np

import concourse.bass as bass
import concourse.tile as tile
from concourse import bacc, mybir
from concourse.masks import make_identity

F32 = mybir.dt.float32
BF16 = mybir.dt.bfloat16
AF = mybir.ActivationFunctionType
ALU = mybir.AluOpType

P = 128
N_CORES = 8
B_LOC = 4            # examples per core
TX = 1024
E = 1024             # enc_hid
A = 1024             # attn
D = 1024             # dec_hid
EO = E // P          # e-chunks
AO = A // P          # a-chunks
DO = D // P          # d-chunks
NT = TX // 512       # t-tiles for energy free dim
MASK_PEN = 100.0     # additive penalty on masked scores (|s| <= ~26)


def build_nc():
    nc = bacc.Bacc(
        "TRN2", target_bir_lowering=False, debug=False, num_devices=N_CORES
    )
    encT = nc.dram_tensor("encT", [B_LOC, E, TX], BF16, kind="ExternalInput").ap()
    enc3 = nc.dram_tensor("enc3", [TX, E], BF16, kind="ExternalInput").ap()
    w_encT = nc.dram_tensor("w_encT", [E, A], BF16, kind="ExternalInput").ap()
    w_decT = nc.dram_tensor("w_decT", [D, A], BF16, kind="ExternalInput").ap()
    dec_hT = nc.dram_tensor("dec_hT", [D, B_LOC], BF16, kind="ExternalInput").ap()
    v_col = nc.dram_tensor("v_col", [A, 1], BF16, kind="ExternalInput").ap()
    wb8 = nc.dram_tensor("wb8", [P, AO], F32, kind="ExternalInput").ap()
    mb = nc.dram_tensor("mb", [B_LOC, TX], BF16, kind="ExternalInput").ap()
    ctx_out = nc.dram_tensor("context", [B_LOC, E], F32, kind="ExternalOutput").ap()
    alpha_out = nc.dram_tensor("alpha", [B_LOC, TX], F32, kind="ExternalOutput").ap()

    with tile.TileContext(nc) as tc, ExitStack() as ctx:
        const = ctx.enter_context(tc.tile_pool(name="const", bufs=1))
        big = ctx.enter_context(tc.tile_pool(name="big", bufs=6))
        en_pool = ctx.enter_context(tc.tile_pool(name="energy", bufs=6))
        small = ctx.enter_context(tc.tile_pool(name="small", bufs=2))
        rowp = ctx.enter_context(tc.tile_pool(name="rows", bufs=2))
        ep_psum = ctx.enter_context(tc.tile_pool(name="ep_ps", bufs=4, space="PSUM"))
        vec_psum = ctx.enter_context(tc.tile_pool(name="vec_ps", bufs=4, space="PSUM"))

        # ---- tiles (all resident for the whole kernel; nothing slot-gated)
        w_encT_sb = const.tile([P, EO, A], BF16)
        encT_tiles = [
            big.tile([P, EO, TX], BF16, tag="big", name=f"encT_sb{b}")
            for b in range(B_LOC)
        ]
        w_decT_sb = big.tile([P, DO, A], BF16, tag="big", name="w_decT_sb")

        dec_hT_sb = const.tile([P, DO, B_LOC], BF16)
        nc.gpsimd.dma_start(
            dec_hT_sb[:], dec_hT.rearrange("(do p) b -> p do b", p=P)
        )
        v_sb = const.tile([P, AO, 1], BF16)
        nc.gpsimd.dma_start(
            v_sb[:], v_col.rearrange("(ao p) one -> p ao one", p=P)
        )
        wb_sb = const.tile([P, AO], F32)
        nc.gpsimd.dma_start(wb_sb[:], wb8[:])
        mb_rows = []
        for b in range(B_LOC):
            mr = small.tile([1, TX], BF16, tag="mbrow", bufs=B_LOC, name=f"mb{b}")
            nc.gpsimd.dma_start(mr[:], mb[b : b + 1, :])
            mb_rows.append(mr)
        ident4 = const.tile([B_LOC, B_LOC], F32)
        make_identity(nc, ident4[:])
        ident128 = const.tile([P, P], F32)
        make_identity(nc, ident128[:])
        ones_row = const.tile([1, P], BF16)
        nc.vector.memset(ones_row[:], 1.0)
        ident1 = const.tile([1, 1], BF16)
        nc.vector.memset(ident1[:], 1.0)

        # ---- preamble loads, NEED order. b=0's first pass is pure DMA
        # pacing (~200GB/s aggregate over two queues), so the startup-
        # critical 4MB (w_encT + encT0) is spread over FOUR queues. Each
        # compute-engine queue only carries what lands before its first
        # compute op: scalar (tanh at ~20us) gets w_decT-lo + 4 chunks;
        # vector (dp-copy at ~14us) gets 4 chunks and never enough to hit
        # DGE flow-control waits; the rest rides sync/gpsimd.
        for do in range(DO):
            nc.scalar.dma_start(
                w_decT_sb[:, do, 0:512], w_decT[do * P : (do + 1) * P, 0:512]
            )
        # The paired b=0 ramp doesn't need its first tanh until ~30us, so
        # the scalar queue (free after w_decT-lo at ~16us) can carry the
        # two LAST pairs without blocking the ACT stream; w_decT-hi rides
        # sync after the pairs (dec_proj at=1 is deferred to pass ao=2).
        pair_lanes = [
            nc.sync, nc.gpsimd, nc.sync, nc.gpsimd,
            nc.sync, nc.gpsimd, nc.scalar, nc.scalar,
        ]
        for eo in range(EO):
            eng = pair_lanes[eo]
            eng.dma_start(
                w_encT_sb[:, eo], w_encT[eo * P : (eo + 1) * P, :]
            )
            eng.dma_start(
                encT_tiles[0][:, eo], encT[0, eo * P : (eo + 1) * P, :]
            )
        for do in range(DO):
            nc.sync.dma_start(
                w_decT_sb[:, do, 512:1024], w_decT[do * P : (do + 1) * P, 512:1024]
            )
        lanes = [nc.sync, nc.gpsimd]
        lane_i = [0]

        def lane():
            eng = lanes[lane_i[0] % 2]
            lane_i[0] += 1
            return eng

        for b in range(1, B_LOC):
            for eo in range(EO):
                lane().dma_start(
                    encT_tiles[b][:, eo], encT[b, eo * P : (eo + 1) * P, :]
                )
        # natural-layout enc for b=3's PE context (t = to*128 + p on
        # partitions, matching the PE-transposed exp columns)
        enc3_sb = big.tile([P, TX // P, E], BF16, tag="big", name="enc3_sb")
        nc.gpsimd.dma_start(
            enc3_sb[:], enc3.rearrange("(to p) e -> p to e", p=P)
        )

        bias_sb = const.tile([P, AO, B_LOC], F32)
        dp_row = rowp.tile([B_LOC, A], F32, tag="row4k", name="dp_row")

        def finalize_bias(ao_lo, ao_hi):
            # bias[a-part, b] = dec_proj^T + W_b via PE transposes (PE is
            # otherwise DMA-starved this early; no DRAM bounce needed).
            for ao in range(ao_lo, ao_hi):
                tp_ps = vec_psum.tile(
                    [P, B_LOC], F32, tag="vec", name=f"tp_ps{ao}"
                )
                nc.tensor.transpose(
                    tp_ps[:], dp_row[:, ao * P : (ao + 1) * P], ident4[:]
                )
                nc.vector.tensor_scalar_add(
                    bias_sb[:, ao], tp_ps[:], wb_sb[:, ao : ao + 1]
                )

        def dec_proj_pass(at):
            dp_ps = ep_psum.tile([P, 512], F32, tag="ep", name=f"dp_ps{at}")
            for do in range(DO):
                nc.tensor.matmul(
                    dp_ps[:B_LOC, :],
                    lhsT=dec_hT_sb[:, do],
                    rhs=w_decT_sb[:, do, at * 512 : (at + 1) * 512],
                    start=(do == 0),
                    stop=(do == DO - 1),
                )
            nc.vector.tensor_copy(
                dp_row[:, at * 512 : (at + 1) * 512], dp_ps[:B_LOC, :]
            )
            finalize_bias(at * 4, at * 4 + 4)

        # at=0 runs at the head of the PE stream (its w_decT half leads the
        # DMA order); at=1's weights arrive after w_encT+encT0, so that pass
        # is deferred into b=0's energy stream to avoid head-of-line
        # blocking the in-order PE queue.
        dec_proj_pass(0)

        # ---- per-example pipeline -----------------------------------------
        # The PE queue is in-order, so every matmul that waits on a non-PE
        # producer is emitted at least one ao-pass after that producer's
        # input was ready: score matmuls trail their tanh by one pass, and
        # example b's context reduction is emitted inside example b+1's
        # energy stream.
        pend_score = None   # closure emitting the previous pass's score MMs
        pend_ctx = None     # closure emitting the previous example's context
        pend_ctx_tp = None  # its PE transpose + store, deferred further

        for b in range(B_LOC):
            encT_sb = encT_tiles[b]
            sc_ps = [
                vec_psum.tile([1, 512], F32, tag="vec", name=f"sc{b}_{nt}")
                for nt in range(NT)
            ]

            def flush_score_for(ao, ens, sc_ps=sc_ps):
                def fl():
                    for nt in range(NT):
                        nc.tensor.matmul(
                            sc_ps[nt][:],
                            lhsT=v_sb[:, ao],
                            rhs=ens[nt][:],
                            start=(ao == 0),
                            stop=False,
                        )
                return fl

            ao_start = 0
            if b == 0:
                # b=0's first pass is paced by the encT0/w_encT DMA stream
                # (~2 chunks per pass-worth of PE time), so run passes
                # ao=0,1 together eo-outer: each arriving chunk feeds 4
                # matmuls and the PE banks two passes of work during the
                # ramp. Uses all 4 ep PSUM banks.
                eps01 = [
                    [
                        ep_psum.tile(
                            [P, 512], F32, tag="ep", name=f"ep0_{ao}_{nt}"
                        )
                        for nt in range(NT)
                    ]
                    for ao in range(2)
                ]
                for eo in range(EO):
                    for ao in range(2):
                        for nt in range(NT):
                            nc.tensor.matmul(
                                eps01[ao][nt][:],
                                lhsT=w_encT_sb[:, eo, ao * P : (ao + 1) * P],
                                rhs=encT_sb[:, eo, nt * 512 : (nt + 1) * 512],
                                start=(eo == 0),
                                stop=(eo == EO - 1),
                            )
                ens01 = []
                for ao in range(2):
                    ens = []
                    for nt in range(NT):
                        energy = en_pool.tile(
                            [P, 512], BF16, tag="energy", name=f"en0_{ao}_{nt}"
                        )
                        nc.scalar.activation(
                            energy[:], eps01[ao][nt][:], AF.Tanh,
                            bias=bias_sb[:, ao, 0:1],
                        )
                        ens.append(energy)
                    ens01.append(ens)
                flush_score_for(0, ens01[0])()
                pend_score = flush_score_for(1, ens01[1])
                ao_start = 2

            # energy^T tiles + deferred score accumulation. eo-outer /
            # nt-inner so both nt-halves reuse each stationary weight chunk.
            for ao in range(ao_start, AO):
                # nt halves run SEQUENTIALLY (not interleaved): walrus emits
                # one LDWEIGHTS per matmul regardless, so there is no
                # weight-reuse win from pairing -- but finishing nt=0's
                # accumulation at mid-pass lets its tanh start ~1.7us
                # earlier, which keeps the deferred score matmuls and the
                # final softmax chain off the critical path.
                eps = [
                    ep_psum.tile([P, 512], F32, tag="ep", name=f"ep{b}_{ao}_{nt}")
                    for nt in range(NT)
                ]
                ens = []
                for nt in range(NT):
                    for eo in range(EO):
                        nc.tensor.matmul(
                            eps[nt][:],
                            lhsT=w_encT_sb[:, eo, ao * P : (ao + 1) * P],
                            rhs=encT_sb[:, eo, nt * 512 : (nt + 1) * 512],
                            start=(eo == 0),
                            stop=(eo == EO - 1),
                        )
                    energy = en_pool.tile(
                        [P, 512], BF16, tag="energy", name=f"en{b}_{ao}_{nt}"
                    )
                    nc.scalar.activation(
                        energy[:], eps[nt][:], AF.Tanh,
                        bias=bias_sb[:, ao, b : b + 1],
                    )
                    ens.append(energy)
                if pend_score is not None:
                    pend_score()

                def flush_score(ao=ao, ens=ens, sc_ps=sc_ps):
                    for nt in range(NT):
                        nc.tensor.matmul(
                            sc_ps[nt][:],
                            lhsT=v_sb[:, ao],
                            rhs=ens[nt][:],
                            start=(ao == 0),
                            stop=False,
                        )

                pend_score = flush_score
                if b == 0 and ao == 2:
                    dec_proj_pass(1)
                if pend_ctx is not None and ao == 1:
                    pend_ctx()
                    pend_ctx = None
                if pend_ctx_tp is not None and ao == 5:
                    pend_ctx_tp()
                    pend_ctx_tp = None
            pend_score()
            pend_score = None

            # masked softmax. Scores are bounded (|s| <= sum|v| ~ 26 since
            # |tanh| <= 1) so exp needs no max shift -- softmax is
            # shift-invariant. The -100 mask penalty rides into the score
            # PSUM as one extra K=1 accumulation step (ones^T @ mb_row), so
            # exp reads the PSUM directly; it flushes masked lanes to 0.0
            # in bf16 and accumulates the row sum for free via accum_out.
            exp_bf = rowp.tile([1, TX], BF16, tag="erow", name=f"exp{b}")
            asum = small.tile([1, NT], F32, tag="asum", name=f"asum{b}")
            for nt in range(NT):
                hs = slice(nt * 512, (nt + 1) * 512)
                nc.tensor.matmul(
                    sc_ps[nt][:],
                    lhsT=ones_row[:, 0:1],
                    rhs=mb_rows[b][:, hs],
                    start=False,
                    stop=True,
                )
                nc.scalar.activation(
                    exp_bf[:, hs], sc_ps[nt][:], AF.Exp,
                    accum_out=asum[:, nt : nt + 1],
                )

            ssum = small.tile([1, 1], F32, tag="ssum", name=f"ssum{b}")
            nc.vector.tensor_add(
                out=ssum[:], in0=asum[:, 0:1], in1=asum[:, 1:2]
            )
            rsum = small.tile([1, 1], F32, tag="rsum", name=f"rsum{b}")
            nc.vector.reciprocal(rsum[:], ssum[:])
            # fp32 alpha row for the output; bf16 normalized row for the
            # context reduction (broadcast below)
            alpha_row = rowp.tile([1, TX], F32, tag="arow", name=f"alpha{b}")
            nc.vector.tensor_scalar_mul(alpha_row[:], exp_bf[:], rsum[:])
            nc.sync.dma_start(alpha_out[b : b + 1, :], alpha_row[:])

            if b == B_LOC - 1:
                # Last example: the context is the kernel's serial tail, so
                # run it on the PE (shorter critical path than the DVE
                # chain): land t on partitions via 8 PE column-transposes
                # of the exp row (the PE is idle here; ~2us vs ~6us for a
                # DRAM bounce), then 16 accumulating [1,512] matmuls
                # against the natural-layout enc, 1/sum folded into the
                # PSUM evacuation.
                TO = TX // P
                # bf16 PSUM writes must be 4B aligned: pad each transposed
                # column to a 4-byte lane pair and evac with a stride
                expT_ps = vec_psum.tile(
                    [P, TO, 2], BF16, tag="vec", name="expTps"
                )
                for k in range(TO):
                    nc.tensor.transpose(
                        expT_ps[:, k, 0:1],
                        exp_bf[:, k * P : (k + 1) * P],
                        ident1[:],
                    )
                expT = small.tile([P, TO], BF16, tag="expT", name="expT3")
                nc.vector.tensor_copy(expT[:], expT_ps[:, :, 0])

                def emit_ctx_pe(b=b, expT=expT, rsum=rsum):
                    cxs = [
                        vec_psum.tile([1, 512], F32, tag="vec", name=f"cx{et}")
                        for et in range(2)
                    ]
                    for to in range(TO):
                        for et in range(2):
                            nc.tensor.matmul(
                                cxs[et][:],
                                lhsT=expT[:, to : to + 1],
                                rhs=enc3_sb[:, to, et * 512 : (et + 1) * 512],
                                start=(to == 0),
                                stop=(to == TO - 1),
                            )
                    ctx_row = rowp.tile([1, E], F32, tag="row4k", name="ctx3")
                    for et in range(2):
                        nc.vector.tensor_scalar_mul(
                            ctx_row[:, et * 512 : (et + 1) * 512], cxs[et][:],
                            rsum[:],
                        )
                    nc.sync.dma_start(ctx_out[b : b + 1, :], ctx_row[:])

                pend_ctx = emit_ctx_pe
                continue

            alpha_bf = rowp.tile([1, TX], BF16, tag="abrow", name=f"alphabf{b}")
            nc.vector.tensor_scalar_mul(alpha_bf[:], exp_bf[:], rsum[:])

            def emit_ctx(b=b, encT_sb=encT_sb, alpha_bf=alpha_bf):
                nonlocal pend_ctx_tp
                # context^T[e-part, eo] = sum_t alpha[t] encT[e, t] on the
                # vector engine (alpha pre-normalized). alpha is broadcast
                # to all 128 partitions with a K=1 ones-column matmul (the
                # gpsimd partition_broadcast custom op crashes this
                # runtime), then 8 chunk multiplies + one 3D reduce
                # contract over t. A final PE transpose lands the row
                # layout for a clean contiguous store.
                alpha_bc = rowp.tile([P, TX], BF16, tag="abc", name=f"abc{b}")
                for nt in range(NT):
                    hs = slice(nt * 512, (nt + 1) * 512)
                    bc_ps = vec_psum.tile(
                        [P, 512], F32, tag="vec", name=f"bc{b}_{nt}"
                    )
                    nc.tensor.matmul(
                        bc_ps[:], lhsT=ones_row[:], rhs=alpha_bf[:, hs],
                        start=True, stop=True,
                    )
                    nc.vector.tensor_copy(alpha_bc[:, hs], bc_ps[:])
                prod = rowp.tile([P, EO, TX], BF16, tag="prod", name=f"pr{b}")
                for eo in range(EO):
                    nc.vector.tensor_mul(
                        out=prod[:, eo], in0=encT_sb[:, eo], in1=alpha_bc[:]
                    )
                ctxT = small.tile([P, EO], F32, tag="ctxT", name=f"ctxT{b}")
                nc.vector.tensor_reduce(
                    ctxT[:], prod[:], axis=mybir.AxisListType.X, op=ALU.add
                )

                def emit_ctx_tp(b=b, ctxT=ctxT):
                    tp_ps = ep_psum.tile([EO, P], F32, tag="ep", name=f"ctp{b}")
                    nc.tensor.transpose(tp_ps[:], ctxT[:], ident128[:])
                    ctx_row = small.tile(
                        [EO, P], F32, tag="ctxrow", name=f"cr{b}"
                    )
                    nc.vector.tensor_copy(ctx_row[:], tp_ps[:])
                    nc.sync.dma_start(
                        ctx_out[b].rearrange("(eo p) -> eo p", p=P), ctx_row[:]
                    )

                pend_ctx_tp = emit_ctx_tp

            pend_ctx = emit_ctx

        if pend_ctx_tp is not None:
            pend_ctx_tp()
            pend_ctx_tp = None
        pend_ctx()
        if pend_ctx_tp is not None:
            pend_ctx_tp()

    nc.compile()
    return nc


_NC = None


def _get_nc():
    global _NC
    if _NC is None:
        _NC = build_nc()
    return _NC


def make_in_maps(dec_hidden, enc_outputs, mask, W_w, W_b, v_w):
    import ml_dtypes

    BF = ml_dtypes.bfloat16
    dec_hidden = np.asarray(dec_hidden, np.float32)
    enc_outputs = np.asarray(enc_outputs, np.float32)
    W_w = np.asarray(W_w, np.float32)
    W_b = np.asarray(W_b, np.float32)
    v_w = np.asarray(v_w, np.float32)
    mb = ((np.asarray(mask).astype(np.float32) - 1.0) * MASK_PEN).astype(BF)

    encT = np.ascontiguousarray(enc_outputs.transpose(0, 2, 1).astype(BF))
    w_encT = np.ascontiguousarray(W_w[:, D:].T.astype(BF))
    w_decT = np.ascontiguousarray(W_w[:, :D].T.astype(BF))
    wb8 = np.ascontiguousarray(W_b.reshape(AO, P).T)
    v_col = np.ascontiguousarray(v_w.reshape(A, 1).astype(BF))

    in_maps = []
    for c in range(N_CORES):
        sl = slice(B_LOC * c, B_LOC * (c + 1))
        in_maps.append(
            {
                "encT": encT[sl],
                "enc3": np.ascontiguousarray(
                    enc_outputs[B_LOC * c + 3].astype(BF)
                ),
                "w_encT": w_encT,
                "w_decT": w_decT,
                "dec_hT": np.ascontiguousarray(dec_hidden[sl].T.astype(BF)),
                "v_col": v_col,
                "wb8": wb8,
                "mb": np.ascontiguousarray(mb[sl]),
            }
        )
    return in_maps


def kernel(dec_hidden, enc_outputs, mask, W_w, W_b, v_w):
    from concourse.bass_utils import run_bass_kernel_spmd

    assert enc_outputs.shape == (N_CORES * B_LOC, TX, E), enc_outputs.shape
    nc = _get_nc()
    in_maps = make_in_maps(dec_hidden, enc_outputs, mask, W_w, W_b, v_w)
    res = run_bass_kernel_spmd(nc, in_maps, list(range(N_CORES))).results
    context = np.concatenate([res[c]["context"] for c in range(N_CORES)], axis=0)
    alpha = np.concatenate([res[c]["alpha"] for c in range(N_CORES)], axis=0)
    return context, alpha


# revision 53
# speedup vs baseline: 1.1100x; 1.1100x over previous
"""Bahdanau attention kernel for Trainium2 (Bass/Tile), 8-core data-parallel.

Problem shapes: B=32, Tx=1024, enc_hid=dec_hid=attn=1024, fp32 in/out.

Math (per example b):
  dec_proj = W_dec @ dec_hidden[b]                 [attn]
  energy^T[a, t] = tanh(sum_e W_enc[a,e] enc[b,t,e] + dec_proj[a] + W_b[a])
  scores[t] = sum_a v[a] energy^T[a, t]
  alpha = softmax(mask(scores))
  context[e] = sum_t alpha[t] enc[b,t,e]

Sharding: batch B split 4 examples per core across 8 cores; weights replicated.

All matmul operands are bf16: on TRN2 silicon a 512-moving-row fp32r matmul
measures ~394ns while bf16 measures ~260ns steady-state, and bf16 halves HBM
traffic and SBUF footprint. PSUM accumulation stays fp32. Measured fp22
end-to-end rel err was 2e-4; bf16's 8-bit mantissa puts it at ~3e-3, inside
the 2e-2 gate.

Masking is folded into the scores as an additive -100 penalty before a
single bf16 exp: masked lanes give exp(s-100) < 1e-40 which flushes to 0.0
in bf16 -- exactly the reference's masked_fill(-1e9) softmax behavior.

The context reduction runs on the VECTOR engine against the already-resident
encT tiles: alpha (bf16, pre-normalized) is partition-broadcast to [128, Tx]
and tensor_tensor_reduce contracts over t per 128-wide e-chunk. This deletes
the whole natural-layout enc DMA stream (8MB/core) and 64 context matmuls,
and leaves every SBUF tile resident for the kernel's lifetime (no slot
gating -> mid-kernel DMAs can never head-of-line block a compute queue).

Layouts per core (host-side preprocessing in kernel()):
  encT  [4, E, Tx]  bf16  enc transposed  -> e on partitions (energy rhs)
  w_encT [E, A], w_decT [D, A] bf16       transposed nn.Linear weights
  dec_hT [D, 4], v_col [A, 1] bf16, wb8 [128, 8] f32, mb [4, Tx] f32
"""

from contextlib import ExitStack

import numpy as np

import concourse.bass as bass
import concourse.tile as tile
from concourse import bacc, mybir
from concourse.masks import make_identity

F32 = mybir.dt.float32
BF16 = mybir.dt.bfloat16
AF = mybir.ActivationFunctionType
ALU = mybir.AluOpType

P = 128
N_CORES = 8
B_LOC = 4            # examples per core
TX = 1024
E = 1024             # enc_hid
A = 1024             # attn
D = 1024             # dec_hid
EO = E // P          # e-chunks
AO = A // P          # a-chunks
DO = D // P          # d-chunks
NT = TX // 512       # t-tiles for energy free dim
MASK_PEN = 100.0     # additive penalty on masked scores (|s| <= ~26)


def build_nc():
    nc = bacc.Bacc(
        "TRN2", target_bir_lowering=False, debug=False, num_devices=N_CORES
    )
    encT = nc.dram_tensor("encT", [B_LOC, E, TX], BF16, kind="ExternalInput").ap()
    enc3 = nc.dram_tensor("enc3", [TX, E], BF16, kind="ExternalInput").ap()
    w_encT = nc.dram_tensor("w_encT", [E, A], BF16, kind="ExternalInput").ap()
    w_decT = nc.dram_tensor("w_decT", [D, A], BF16, kind="ExternalInput").ap()
    dec_hT = nc.dram_tensor("dec_hT", [D, B_LOC], BF16, kind="ExternalInput").ap()
    v_col = nc.dram_tensor("v_col", [A, 1], BF16, kind="ExternalInput").ap()
    wb8 = nc.dram_tensor("wb8", [P, AO], F32, kind="ExternalInput").ap()
    mb = nc.dram_tensor("mb", [B_LOC, TX], BF16, kind="ExternalInput").ap()
    ctx_out = nc.dram_tensor("context", [B_LOC, E], F32, kind="ExternalOutput").ap()
    alpha_out = nc.dram_tensor("alpha", [B_LOC, TX], F32, kind="ExternalOutput").ap()

    with tile.TileContext(nc) as tc, ExitStack() as ctx:
        const = ctx.enter_context(tc.tile_pool(name="const", bufs=1))
        big = ctx.enter_context(tc.tile_pool(name="big", bufs=6))
        en_pool = ctx.enter_context(tc.tile_pool(name="energy", bufs=6))
        small = ctx.enter_context(tc.tile_pool(name="small", bufs=2))
        rowp = ctx.enter_context(tc.tile_pool(name="rows", bufs=2))
        ep_psum = ctx.enter_context(tc.tile_pool(name="ep_ps", bufs=4, space="PSUM"))
        vec_psum = ctx.enter_context(tc.tile_pool(name="vec_ps", bufs=4, space="PSUM"))

        # ---- tiles (all resident for the whole kernel; nothing slot-gated)
        w_encT_sb = const.tile([P, EO, A], BF16)
        encT_tiles = [
            big.tile([P, EO, TX], BF16, tag="big", name=f"encT_sb{b}")
            for b in range(B_LOC)
        ]
        w_decT_sb = big.tile([P, DO, A], BF16, tag="big", name="w_decT_sb")

        dec_hT_sb = const.tile([P, DO, B_LOC], BF16)
        nc.gpsimd.dma_start(
            dec_hT_sb[:], dec_hT.rearrange("(do p) b -> p do b", p=P)
        )
        v_sb = const.tile([P, AO, 1], BF16)
        nc.gpsimd.dma_start(
            v_sb[:], v_col.rearrange("(ao p) one -> p ao one", p=P)
        )
        wb_sb = const.tile([P, AO], F32)
        nc.gpsimd.dma_start(wb_sb[:], wb8[:])
        mb_rows = []
        for b in range(B_LOC):
            mr = small.tile([1, TX], BF16, tag="mbrow", bufs=B_LOC, name=f"mb{b}")
            nc.gpsimd.dma_start(mr[:], mb[b : b + 1, :])
            mb_rows.append(mr)
        ident4 = const.tile([B_LOC, B_LOC], F32)
        make_identity(nc, ident4[:])
        ident128 = const.tile([P, P], F32)
        make_identity(nc, ident128[:])
        ones_row = const.tile([1, P], BF16)
        nc.vector.memset(ones_row[:], 1.0)
        ident1 = const.tile([1, 1], BF16)
        nc.vector.memset(ident1[:], 1.0)

        # ---- preamble loads, NEED order. b=0's first pass is pure DMA
        # pacing (~200GB/s aggregate over two queues), so the startup-
        # critical 4MB (w_encT + encT0) is spread over FOUR queues. Each
        # compute-engine queue only carries what lands before its first
        # compute op: scalar (tanh at ~20us) gets w_decT-lo + 4 chunks;
        # vector (dp-copy at ~14us) gets 4 chunks and never enough to hit
        # DGE flow-control waits; the rest rides sync/gpsimd.
        for do in range(DO):
            nc.scalar.dma_start(
                w_decT_sb[:, do, 0:512], w_decT[do * P : (do + 1) * P, 0:512]
            )
        pair_lanes = [nc.sync, nc.gpsimd]
        for eo in range(EO):
            eng = pair_lanes[eo % 2]
            eng.dma_start(
                w_encT_sb[:, eo], w_encT[eo * P : (eo + 1) * P, :]
            )
            eng.dma_start(
                encT_tiles[0][:, eo], encT[0, eo * P : (eo + 1) * P, :]
            )
        for do in range(DO):
            nc.scalar.dma_start(
                w_decT_sb[:, do, 512:1024], w_decT[do * P : (do + 1) * P, 512:1024]
            )
        lanes = [nc.sync, nc.gpsimd]
        lane_i = [0]

        def lane():
            eng = lanes[lane_i[0] % 2]
            lane_i[0] += 1
            return eng

        for b in range(1, B_LOC):
            for eo in range(EO):
                lane().dma_start(
                    encT_tiles[b][:, eo], encT[b, eo * P : (eo + 1) * P, :]
                )
        # natural-layout enc for b=3's PE context (t = to*128 + p on
        # partitions, matching the PE-transposed exp columns)
        enc3_sb = big.tile([P, TX // P, E], BF16, tag="big", name="enc3_sb")
        nc.gpsimd.dma_start(
            enc3_sb[:], enc3.rearrange("(to p) e -> p to e", p=P)
        )

        bias_sb = const.tile([P, AO, B_LOC], F32)
        dp_row = rowp.tile([B_LOC, A], F32, tag="row4k", name="dp_row")

        def finalize_bias(ao_lo, ao_hi):
            # bias[a-part, b] = dec_proj^T + W_b via PE transposes (PE is
            # otherwise DMA-starved this early; no DRAM bounce needed).
            for ao in range(ao_lo, ao_hi):
                tp_ps = vec_psum.tile(
                    [P, B_LOC], F32, tag="vec", name=f"tp_ps{ao}"
                )
                nc.tensor.transpose(
                    tp_ps[:], dp_row[:, ao * P : (ao + 1) * P], ident4[:]
                )
                nc.vector.tensor_scalar_add(
                    bias_sb[:, ao], tp_ps[:], wb_sb[:, ao : ao + 1]
                )

        def dec_proj_pass(at):
            dp_ps = ep_psum.tile([P, 512], F32, tag="ep", name=f"dp_ps{at}")
            for do in range(DO):
                nc.tensor.matmul(
                    dp_ps[:B_LOC, :],
                    lhsT=dec_hT_sb[:, do],
                    rhs=w_decT_sb[:, do, at * 512 : (at + 1) * 512],
                    start=(do == 0),
                    stop=(do == DO - 1),
                )
            nc.vector.tensor_copy(
                dp_row[:, at * 512 : (at + 1) * 512], dp_ps[:B_LOC, :]
            )
            finalize_bias(at * 4, at * 4 + 4)

        # at=0 runs at the head of the PE stream (its w_decT half leads the
        # DMA order); at=1's weights arrive after w_encT+encT0, so that pass
        # is deferred into b=0's energy stream to avoid head-of-line
        # blocking the in-order PE queue.
        dec_proj_pass(0)

        # ---- per-example pipeline -----------------------------------------
        # The PE queue is in-order, so every matmul that waits on a non-PE
        # producer is emitted at least one ao-pass after that producer's
        # input was ready: score matmuls trail their tanh by one pass, and
        # example b's context reduction is emitted inside example b+1's
        # energy stream.
        pend_score = None   # closure emitting the previous pass's score MMs
        pend_ctx = None     # closure emitting the previous example's context
        pend_ctx_tp = None  # its PE transpose + store, deferred further

        for b in range(B_LOC):
            encT_sb = encT_tiles[b]
            sc_ps = [
                vec_psum.tile([1, 512], F32, tag="vec", name=f"sc{b}_{nt}")
                for nt in range(NT)
            ]

            def flush_score_for(ao, ens, sc_ps=sc_ps):
                def fl():
                    for nt in range(NT):
                        nc.tensor.matmul(
                            sc_ps[nt][:],
                            lhsT=v_sb[:, ao],
                            rhs=ens[nt][:],
                            start=(ao == 0),
                            stop=False,
                        )
                return fl

            ao_start = 0
            if b == 0:
                # b=0's first pass is paced by the encT0/w_encT DMA stream
                # (~2 chunks per pass-worth of PE time), so run passes
                # ao=0,1 together eo-outer: each arriving chunk feeds 4
                # matmuls and the PE banks two passes of work during the
                # ramp. Uses all 4 ep PSUM banks.
                eps01 = [
                    [
                        ep_psum.tile(
                            [P, 512], F32, tag="ep", name=f"ep0_{ao}_{nt}"
                        )
                        for nt in range(NT)
                    ]
                    for ao in range(2)
                ]
                for eo in range(EO):
                    for ao in range(2):
                        for nt in range(NT):
                            nc.tensor.matmul(
                                eps01[ao][nt][:],
                                lhsT=w_encT_sb[:, eo, ao * P : (ao + 1) * P],
                                rhs=encT_sb[:, eo, nt * 512 : (nt + 1) * 512],
                                start=(eo == 0),
                                stop=(eo == EO - 1),
                            )
                ens01 = []
                for ao in range(2):
                    ens = []
                    for nt in range(NT):
                        energy = en_pool.tile(
                            [P, 512], BF16, tag="energy", name=f"en0_{ao}_{nt}"
                        )
                        nc.scalar.activation(
                            energy[:], eps01[ao][nt][:], AF.Tanh,
                            bias=bias_sb[:, ao, 0:1],
                        )
                        ens.append(energy)
                    ens01.append(ens)
                # dec_proj at=1 fills the PE while the first tanhs drain
                dec_proj_pass(1)
                flush_score_for(0, ens01[0])()
                pend_score = flush_score_for(1, ens01[1])
                ao_start = 2

            # energy^T tiles + deferred score accumulation. eo-outer /
            # nt-inner so both nt-halves reuse each stationary weight chunk.
            for ao in range(ao_start, AO):
                # nt halves run SEQUENTIALLY (not interleaved): walrus emits
                # one LDWEIGHTS per matmul regardless, so there is no
                # weight-reuse win from pairing -- but finishing nt=0's
                # accumulation at mid-pass lets its tanh start ~1.7us
                # earlier, which keeps the deferred score matmuls and the
                # final softmax chain off the critical path.
                eps = [
                    ep_psum.tile([P, 512], F32, tag="ep", name=f"ep{b}_{ao}_{nt}")
                    for nt in range(NT)
                ]
                ens = []
                for nt in range(NT):
                    for eo in range(EO):
                        nc.tensor.matmul(
                            eps[nt][:],
                            lhsT=w_encT_sb[:, eo, ao * P : (ao + 1) * P],
                            rhs=encT_sb[:, eo, nt * 512 : (nt + 1) * 512],
                            start=(eo == 0),
                            stop=(eo == EO - 1),
                        )
                    energy = en_pool.tile(
                        [P, 512], BF16, tag="energy", name=f"en{b}_{ao}_{nt}"
                    )
                    nc.scalar.activation(
                        energy[:], eps[nt][:], AF.Tanh,
                        bias=bias_sb[:, ao, b : b + 1],
                    )
                    ens.append(energy)
                if pend_score is not None:
                    pend_score()

                def flush_score(ao=ao, ens=ens, sc_ps=sc_ps):
                    for nt in range(NT):
                        nc.tensor.matmul(
                            sc_ps[nt][:],
                            lhsT=v_sb[:, ao],
                            rhs=ens[nt][:],
                            start=(ao == 0),
                            stop=False,
                        )

                pend_score = flush_score
                if pend_ctx is not None and ao == 1:
                    pend_ctx()
                    pend_ctx = None
                if pend_ctx_tp is not None and ao == 5:
                    pend_ctx_tp()
                    pend_ctx_tp = None
            pend_score()
            pend_score = None

            # masked softmax. Scores are bounded (|s| <= sum|v| ~ 26 since
            # |tanh| <= 1) so exp needs no max shift -- softmax is
            # shift-invariant. The -100 mask penalty rides into the score
            # PSUM as one extra K=1 accumulation step (ones^T @ mb_row), so
            # exp reads the PSUM directly; it flushes masked lanes to 0.0
            # in bf16 and accumulates the row sum for free via accum_out.
            exp_bf = rowp.tile([1, TX], BF16, tag="erow", name=f"exp{b}")
            asum = small.tile([1, NT], F32, tag="asum", name=f"asum{b}")
            for nt in range(NT):
                hs = slice(nt * 512, (nt + 1) * 512)
                nc.tensor.matmul(
                    sc_ps[nt][:],
                    lhsT=ones_row[:, 0:1],
                    rhs=mb_rows[b][:, hs],
                    start=False,
                    stop=True,
                )
                nc.scalar.activation(
                    exp_bf[:, hs], sc_ps[nt][:], AF.Exp,
                    accum_out=asum[:, nt : nt + 1],
                )

            ssum = small.tile([1, 1], F32, tag="ssum", name=f"ssum{b}")
            nc.vector.tensor_add(
                out=ssum[:], in0=asum[:, 0:1], in1=asum[:, 1:2]
            )
            rsum = small.tile([1, 1], F32, tag="rsum", name=f"rsum{b}")
            nc.vector.reciprocal(rsum[:], ssum[:])
            # fp32 alpha row for the output; bf16 normalized row for the
            # context reduction (broadcast below)
            alpha_row = rowp.tile([1, TX], F32, tag="arow", name=f"alpha{b}")
            nc.vector.tensor_scalar_mul(alpha_row[:], exp_bf[:], rsum[:])
            nc.sync.dma_start(alpha_out[b : b + 1, :], alpha_row[:])

            if b == B_LOC - 1:
                # Last example: the context is the kernel's serial tail, so
                # run it on the PE (shorter critical path than the DVE
                # chain): land t on partitions via 8 PE column-transposes
                # of the exp row (the PE is idle here; ~2us vs ~6us for a
                # DRAM bounce), then 16 accumulating [1,512] matmuls
                # against the natural-layout enc, 1/sum folded into the
                # PSUM evacuation.
                TO = TX // P
                # bf16 PSUM writes must be 4B aligned: pad each transposed
                # column to a 4-byte lane pair and evac with a stride
                expT_ps = vec_psum.tile(
                    [P, TO, 2], BF16, tag="vec", name="expTps"
                )
                for k in range(TO):
                    nc.tensor.transpose(
                        expT_ps[:, k, 0:1],
                        exp_bf[:, k * P : (k + 1) * P],
                        ident1[:],
                    )
                expT = small.tile([P, TO], BF16, tag="expT", name="expT3")
                nc.vector.tensor_copy(expT[:], expT_ps[:, :, 0])

                def emit_ctx_pe(b=b, expT=expT, rsum=rsum):
                    cxs = [
                        vec_psum.tile([1, 512], F32, tag="vec", name=f"cx{et}")
                        for et in range(2)
                    ]
                    for to in range(TO):
                        for et in range(2):
                            nc.tensor.matmul(
                                cxs[et][:],
                                lhsT=expT[:, to : to + 1],
                                rhs=enc3_sb[:, to, et * 512 : (et + 1) * 512],
                                start=(to == 0),
                                stop=(to == TO - 1),
                            )
                    ctx_row = rowp.tile([1, E], F32, tag="row4k", name="ctx3")
                    for et in range(2):
                        nc.vector.tensor_scalar_mul(
                            ctx_row[:, et * 512 : (et + 1) * 512], cxs[et][:],
                            rsum[:],
                        )
                    nc.sync.dma_start(ctx_out[b : b + 1, :], ctx_row[:])

                pend_ctx = emit_ctx_pe
                continue

            alpha_bf = rowp.tile([1, TX], BF16, tag="abrow", name=f"alphabf{b}")
            nc.vector.tensor_scalar_mul(alpha_bf[:], exp_bf[:], rsum[:])

            def emit_ctx(b=b, encT_sb=encT_sb, alpha_bf=alpha_bf):
                nonlocal pend_ctx_tp
                # context^T[e-part, eo] = sum_t alpha[t] encT[e, t] on the
                # vector engine (alpha pre-normalized). alpha is broadcast
                # to all 128 partitions with a K=1 ones-column matmul (the
                # gpsimd partition_broadcast custom op crashes this
                # runtime), then 8 chunk multiplies + one 3D reduce
                # contract over t. A final PE transpose lands the row
                # layout for a clean contiguous store.
                alpha_bc = rowp.tile([P, TX], BF16, tag="abc", name=f"abc{b}")
                for nt in range(NT):
                    hs = slice(nt * 512, (nt + 1) * 512)
                    bc_ps = vec_psum.tile(
                        [P, 512], F32, tag="vec", name=f"bc{b}_{nt}"
                    )
                    nc.tensor.matmul(
                        bc_ps[:], lhsT=ones_row[:], rhs=alpha_bf[:, hs],
                        start=True, stop=True,
                    )
                    nc.vector.tensor_copy(alpha_bc[:, hs], bc_ps[:])
                prod = rowp.tile([P, EO, TX], BF16, tag="prod", name=f"pr{b}")
                for eo in range(EO):
                    nc.vector.tensor_mul(
                        out=prod[:, eo], in0=encT_sb[:, eo], in1=alpha_bc[:]
                    )
                ctxT = small.tile([P, EO], F32, tag="ctxT", name=f"ctxT{b}")
                nc.vector.tensor_reduce(
                    ctxT[:], prod[:], axis=mybir.AxisListType.X, op=ALU.add
                )

                def emit_ctx_tp(b=b, ctxT=ctxT):
                    tp_ps = ep_psum.tile([EO, P], F32, tag="ep", name=f"ctp{b}")
                    nc.tensor.transpose(tp_ps[:], ctxT[:], ident128[:])
                    ctx_row = small.tile(
                        [EO, P], F32, tag="ctxrow", name=f"cr{b}"
                    )
                    nc.vector.tensor_copy(ctx_row[:], tp_ps[:])
                    nc.sync.dma_start(
                        ctx_out[b].rearrange("(eo p) -> eo p", p=P), ctx_row[:]
                    )

                pend_ctx_tp = emit_ctx_tp

            pend_ctx = emit_ctx

        if pend_ctx_tp is not None:
            pend_ctx_tp()
            pend_ctx_tp = None
        pend_ctx()
        if pend_ctx_tp is not None:
            pend_ctx_tp()

    nc.compile()
    return nc


_NC = None


def _get_nc():
    global _NC
    if _NC is None:
        _NC = build_nc()
    return _NC


def make_in_maps(dec_hidden, enc_outputs, mask, W_w, W_b, v_w):
    import ml_dtypes

    BF = ml_dtypes.bfloat16
    dec_hidden = np.asarray(dec_hidden, np.float32)
    enc_outputs = np.asarray(enc_outputs, np.float32)
    W_w = np.asarray(W_w, np.float32)
    W_b = np.asarray(W_b, np.float32)
    v_w = np.asarray(v_w, np.float32)
    mb = ((np.asarray(mask).astype(np.float32) - 1.0) * MASK_PEN).astype(BF)

    encT = np.ascontiguousarray(enc_outputs.transpose(0, 2, 1).astype(BF))
    w_encT = np.ascontiguousarray(W_w[:, D:].T.astype(BF))
    w_decT = np.ascontiguousarray(W_w[:, :D].T.astype(BF))
    wb8 = np.ascontiguousarray(W_b.reshape(AO, P).T)
    v_col = np.ascontiguousarray(v_w.reshape(A, 1).astype(BF))

    in_maps = []
    for c in range(N_CORES):
        sl = slice(B_LOC * c, B_LOC * (c + 1))
        in_maps.append(
            {
                "encT": encT[sl],
                "enc3": np.ascontiguousarray(
                    enc_outputs[B_LOC * c + 3].astype(BF)
                ),
                "w_encT": w_encT,
                "w_decT": w_decT,
                "dec_hT": np.ascontiguousarray(dec_hidden[sl].T.astype(BF)),
                "v_col": v_col,
                "wb8": wb8,
                "mb": np.ascontiguousarray(mb[sl]),
            }
        )
    return in_maps


def kernel(dec_hidden, enc_outputs, mask, W_w, W_b, v_w):
    from concourse.bass_utils import run_bass_kernel_spmd

    assert enc_outputs.shape == (N_CORES * B_LOC, TX, E), enc_outputs.shape
    nc = _get_nc()
    in_maps = make_in_maps(dec_hidden, enc_outputs, mask, W_w, W_b, v_w)
    res = run_bass_kernel_spmd(nc, in_maps, list(range(N_CORES))).results
    context = np.concatenate([res[c]["context"] for c in range(N_CORES)], axis=0)
    alpha = np.concatenate([res[c]["alpha"] for c in range(N_CORES)], axis=0)
    return context, alpha
